# revision 25
# baseline (speedup 1.0000x reference)
"""Trainium2 Bass kernel for MC-sampled cross-entropy-with-variance loss.

Computes mean over (s, b, h, w) of
    nll = logsumexp_c(mean + exp(0.5*log_var)*eps[s]) - logit[label]
distributed over 8 NeuronCores by sharding the H*W pixel axis.

Structure (per core):
- partitions pack (chunk j, class c): p = j*19 + c, chunks of F=2048
  pixels, regions of g chunks ((6,6,4) per image).
- per (region, sample): DVE computes logits x = eps*std + mean in two
  2x-mode tensor_tensor ops on [114,2048] tiles; ACT computes
  e1 = exp(x); PE sums exp over classes via per-sample selector
  matmuls into PSUM rows 6s+j (two [64,1024] psum tiles per region),
  and also accumulates P = sum_s eps via identity matmuls (label term:
  sum_s logit[label] = 10*mean[lab] + std[lab]*P[lab]).
- ln() reads the class-sum PSUM directly with a free-axis accumulator.
- label gather uses host-prepared masked tensors u = onehot*std and
  g = onehot*mean; the u*P dots are deferred and batched per image so
  the in-order DVE queue never waits on the PE->ACT psum round-trip.
- ln() is likewise deferred: class-sum PSUM is copied (bf16) into a
  per-image collect buffer and a single Ln pass runs per image, so the
  ACT exp/ln table sets switch 8 times instead of 24.
- eps DMAs go mostly to the gpsimd SWDGE ring (sprays across ~10 SDMA
  engines); sync/scalar HWDGE rings (~90 GB/s each) carry the rest.
"""

import numpy as np
import ml_dtypes

import concourse.bass as bass
import concourse.bacc as bacc
import concourse.mybir as mybir
from concourse import tile
from concourse.bass_interp import get_hw_module
from concourse.bass_utils import run_bass_kernel_spmd
from concourse.mybir import AluOpType as Alu
from concourse.mybir import ActivationFunctionType as Act

# ---------------------------------------------------------------- sizes
S, B, C, H, W = 10, 4, 19, 512, 512
HW = H * W
NCORES = 8
SLAB = HW // NCORES          # pixels per (core, b) = 32768
F = 2048                     # free-dim pixels per chunk (DMA/DVE tile)
FH = 1024                    # per-instance column half (PSUM limit)
REGIONS = [(6, 0), (6, 12288), (4, 24576)]
F32 = mybir.dt.float32
BF16 = mybir.dt.bfloat16


def _rap(handle, base, poff, g):
    """Chunk-outer region AP: partitions iterate (chunk j, class c)."""
    return bass.AP(tensor=handle, offset=base + poff,
                   ap=[[F, g], [SLAB, C], [1, F]])


def build_program():
    nc = bacc.Bacc("TRN2", target_bir_lowering=False, debug=False,
                   num_devices=NCORES)

    eps_h = nc.dram_tensor("eps_s", [S, B, C, SLAB], BF16, kind="ExternalInput")
    sd_h = nc.dram_tensor("sd_s", [B, C, SLAB], BF16, kind="ExternalInput")
    mean_h = nc.dram_tensor("mean_s", [B, C, SLAB], BF16, kind="ExternalInput")
    u_h = nc.dram_tensor("u_s", [B, C, SLAB], BF16, kind="ExternalInput")
    g_h = nc.dram_tensor("g_s", [B, C, SLAB], BF16, kind="ExternalInput")
    sel6_h = nc.dram_tensor("sel6", [114, S * 64], BF16, kind="ExternalInput")
    sel4_h = nc.dram_tensor("sel4", [76, S * 64], BF16, kind="ExternalInput")
    id_h = nc.dram_tensor("id114", [114, 114], BF16, kind="ExternalInput")
    lse_h = nc.dram_tensor("lse_out", [60, 1], F32, kind="ExternalOutput")
    lab_h = nc.dram_tensor("lab_out", [114, 1], F32, kind="ExternalOutput")
    ohm_h = nc.dram_tensor("ohm_out", [114, 1], F32, kind="ExternalOutput")

    with tile.TileContext(nc) as tc:
        with (
            tc.tile_pool(name="consts", bufs=1) as consts,
            tc.tile_pool(name="meta", bufs=2) as meta,
            tc.tile_pool(name="epsp", bufs=8) as eps_pool,
            tc.tile_pool(name="e1p", bufs=6) as e1_pool,
            tc.tile_pool(name="tp", bufs=4) as t_pool,
            tc.tile_pool(name="coll", bufs=2) as coll_pool,
            tc.tile_pool(name="post", bufs=2) as post,
            tc.tile_pool(name="accp", bufs=1) as acc_pool,
            tc.tile_pool(name="psum", bufs=2, space="PSUM") as psum_pool,
        ):
            sel6_t = consts.tile([114, S * 64], BF16, tag="sel6")
            nc.sync.dma_start(out=sel6_t, in_=sel6_h.ap())
            sel4_t = consts.tile([76, S * 64], BF16, tag="sel4")
            nc.scalar.dma_start(out=sel4_t, in_=sel4_h.ap())
            id_sb = consts.tile([114, 114], BF16, tag="id114")
            nc.sync.dma_start(out=id_sb, in_=id_h.ap())
            sel6_sb = [sel6_t[:, s * 64:(s + 1) * 64] for s in range(S)]
            sel4_sb = [sel4_t[:, s * 64:(s + 1) * 64] for s in range(S)]

            acc_lse = acc_pool.tile([60, 1], F32)
            nc.vector.memset(acc_lse, 0.0)
            acc_lab = acc_pool.tile([114, 1], F32)
            nc.vector.memset(acc_lab, 0.0)
            acc_ohm = acc_pool.tile([114, 1], F32)
            nc.vector.memset(acc_ohm, 0.0)

            # gpsimd's SWDGE sprays across ~10 SDMA engines; the two
            # HWDGE rings (sync/scalar) cap near 90 GB/s each
            dma_rr = [nc.gpsimd, nc.gpsimd, nc.gpsimd, nc.sync,
                      nc.gpsimd, nc.gpsimd, nc.gpsimd, nc.scalar,
                      nc.gpsimd, nc.sync]

            for b in range(B):
                # per-image collect buffer for deferred ln: one [60, 1024]
                # block per instance (region-half); g4 tail rows get 1.0
                collect = coll_pool.tile([60, 6 * FH], BF16, tag="coll")
                nc.vector.memset(collect[:60, 4 * FH:6 * FH], 1.0)
                u_coll = coll_pool.tile([114, 3 * F], BF16, tag="ucoll")
                nc.vector.memset(u_coll[:, 2 * F:3 * F], 0.0)
                p_coll = coll_pool.tile([114, 3 * F], BF16, tag="pcoll")
                nc.vector.memset(p_coll[:, 2 * F:3 * F], 0.0)

                for r, (g, poff) in enumerate(REGIONS):
                    p_ = g * C           # 114 or 76
                    rows = g * S         # 60 or 40
                    sel_sb = sel6_sb if g == 6 else sel4_sb
                    base = b * C * SLAB

                    sd_t = meta.tile([114, F], BF16, tag="sd")
                    nc.sync.dma_start(out=sd_t[:p_, :],
                                      in_=_rap(sd_h, base, poff, g))
                    mean_t = meta.tile([114, F], BF16, tag="mean")
                    nc.scalar.dma_start(out=mean_t[:p_, :],
                                        in_=_rap(mean_h, base, poff, g))
                    nc.gpsimd.dma_start(out=u_coll[:p_, r * F:(r + 1) * F],
                                        in_=_rap(u_h, base, poff, g))
                    g_t = meta.tile([114, F], BF16, tag="g")
                    nc.gpsimd.dma_start(out=g_t[:p_, :],
                                        in_=_rap(g_h, base, poff, g))
                    dmo = meta.tile([114, F], BF16, tag="dmo")
                    gp = post.tile([114, 1], F32, tag="gp")
                    nc.vector.tensor_scalar(
                        dmo[:p_, :], g_t[:p_, :], 1.0, None,
                        Alu.mult, Alu.add, accum_out=gp[:p_])
                    nc.vector.tensor_add(acc_ohm[:p_], acc_ohm[:p_],
                                         gp[:p_])

                    cls_a = psum_pool.tile([64, FH], F32, tag="cls")
                    p_a = psum_pool.tile([114, FH], F32, tag="p")
                    cls_b = psum_pool.tile([64, FH], F32, tag="cls")
                    p_b = psum_pool.tile([114, FH], F32, tag="p")

                    for s in range(S):
                        et = eps_pool.tile([114, F], BF16, tag="et")
                        dma_rr[s].dma_start(
                            out=et[:p_, :],
                            in_=_rap(eps_h, (s * B + b) * C * SLAB,
                                     poff, g))
                        t_t = t_pool.tile([114, F], BF16, tag="t")
                        nc.vector.tensor_mul(t_t[:p_, :], et[:p_, :],
                                             sd_t[:p_, :])
                        x_t = t_pool.tile([114, F], BF16, tag="x")
                        nc.vector.tensor_add(x_t[:p_, :], t_t[:p_, :],
                                             mean_t[:p_, :])
                        e1 = e1_pool.tile([114, F], BF16, tag="e1")
                        nc.scalar.activation(e1[:p_, :], x_t[:p_, :],
                                             Act.Exp)
                        # PE: 4 cls (same sel for both halves) + 4 P
                        for m in range(4):
                            sl = slice(m * 512, (m + 1) * 512)
                            pp = slice((m % 2) * 512, (m % 2) * 512 + 512)
                            cls_ps = cls_a if m < 2 else cls_b
                            nc.tensor.matmul(
                                cls_ps[:, pp], sel_sb[s], e1[:p_, sl],
                                start=(s == 0), stop=(s == S - 1))
                        for m in range(4):
                            sl = slice(m * 512, (m + 1) * 512)
                            pp = slice((m % 2) * 512, (m % 2) * 512 + 512)
                            p_ps = p_a if m < 2 else p_b
                            nc.tensor.matmul(
                                p_ps[:p_, pp], id_sb[:p_, :p_],
                                et[:p_, sl],
                                start=(s == 0), stop=(s == S - 1))

                    for half, (cls_ps, p_ps) in enumerate(
                            [(cls_a, p_a), (cls_b, p_b)]):
                        hs = slice(half * FH, (half + 1) * FH)
                        cs = slice((2 * r + half) * FH,
                                   (2 * r + half + 1) * FH)
                        nc.scalar.copy(collect[:rows, cs],
                                       cls_ps[:rows, :])
                        nc.scalar.copy(
                            p_coll[:p_, r * F + half * FH:
                                   r * F + (half + 1) * FH],
                            p_ps[:p_, :])

                # deferred label dot over the whole image: sum(u * P)
                for rr in range(3):
                    rsl = slice(rr * F, (rr + 1) * F)
                    dm = t_pool.tile([114, F], BF16, tag="t")
                    nc.vector.tensor_mul(dm, p_coll[:, rsl],
                                         u_coll[:, rsl])
                    dm2 = t_pool.tile([114, F], BF16, tag="x")
                    dp = post.tile([114, 1], F32, tag="dp")
                    nc.vector.tensor_scalar(
                        dm2, dm, 1.0, None,
                        Alu.mult, Alu.add, accum_out=dp)
                    nc.vector.tensor_add(acc_lab, acc_lab, dp)

                # one batched ln pass per image (single table-set switch)
                lnb = acc_pool.tile([60, 6 * FH], BF16, tag="lnb")
                lse_p = post.tile([60, 1], F32, tag="lsep")
                nc.scalar.activation(lnb, collect[:60, :], Act.Ln,
                                     accum_out=lse_p)
                nc.vector.tensor_add(acc_lse, acc_lse, lse_p)

            nc.sync.dma_start(out=lse_h.ap(), in_=acc_lse)
            nc.sync.dma_start(out=lab_h.ap(), in_=acc_lab)
            nc.sync.dma_start(out=ohm_h.ap(), in_=acc_ohm)

    nc.compile()
    nc.m = get_hw_module(nc.m)
    return nc


def _consts():
    bf = ml_dtypes.bfloat16
    sel6 = np.zeros((114, S, 64), dtype=bf)
    sel4 = np.zeros((76, S, 64), dtype=bf)
    for s in range(S):
        for p in range(114):
            sel6[p, s, 6 * s + p // C] = 1.0
        for p in range(76):
            sel4[p, s, 4 * s + p // C] = 1.0
    id114 = np.eye(114, dtype=bf)
    return sel6.reshape(114, S * 64), sel4.reshape(76, S * 64), id114


def kernel(mean, log_var, label, eps, _trace=False):
    bf = ml_dtypes.bfloat16
    mean = np.asarray(mean, dtype=np.float32).reshape(B, C, HW)
    log_var = np.asarray(log_var, dtype=np.float32).reshape(B, C, HW)
    lab = np.asarray(label).reshape(B, HW)
    eps_r = np.asarray(eps, dtype=np.float32).reshape(S, B, C, HW)

    std = np.exp(0.5 * log_var)
    oh = (lab[:, None, :] ==
          np.arange(C, dtype=lab.dtype)[None, :, None])
    sd_bf = std.astype(bf)
    mean_bf = mean.astype(bf)
    u_bf = np.where(oh, std, 0.0).astype(bf)
    g_bf = np.where(oh, mean, 0.0).astype(bf)
    eps_bf = eps_r.astype(bf)

    sel6, sel4, id114 = _consts()
    in_maps = []
    for c in range(NCORES):
        lo, hi = c * SLAB, (c + 1) * SLAB
        in_maps.append({
            "eps_s": np.ascontiguousarray(eps_bf[:, :, :, lo:hi]),
            "sd_s": np.ascontiguousarray(sd_bf[:, :, lo:hi]),
            "mean_s": np.ascontiguousarray(mean_bf[:, :, lo:hi]),
            "u_s": np.ascontiguousarray(u_bf[:, :, lo:hi]),
            "g_s": np.ascontiguousarray(g_bf[:, :, lo:hi]),
            "sel6": sel6,
            "sel4": sel4,
            "id114": id114,
        })

    nc = build_program()
    res = run_bass_kernel_spmd(
        nc, in_maps, core_ids=list(range(NCORES)), trace=_trace
    )
    global last_results
    last_results = res

    total = np.float64(0.0)
    for c in range(NCORES):
        total += res.results[c]["lse_out"].astype(np.float64).sum()
        total -= res.results[c]["lab_out"].astype(np.float64).sum()
        total -= 10.0 * res.results[c]["ohm_out"].astype(np.float64).sum()
    loss = total / float(S * B * HW)
    return np.float32(loss)
